# revision 10
# baseline (speedup 1.0000x reference)
"""BLOOM attention block on 8 TRN2 NeuronCores.

Tensor-parallel over heads: core c computes heads 4c..4c+3 for both batches
(8 (b,h) pairs/core). Device math in bf16 with fp32 accumulation:

  phase 1: Q^T/K^T = (Wqkv_qk^T h)  [head-dim on partitions], V = h Wqkv_v
  phase 2: per pair, causal-tiled scores + alibi softmax (stabilized with the
           analytic bias -alibi[q] instead of a max pass), P^T via PE
           transpose, ctx^T = V^T P accumulated in dense-ready layout
  phase 3: out_part = ctx Wd_c (per-core partial, fp32)

Host: shards/casts inputs, then out = residual + bd + sum_c out_part_c.
Self-contained: shapes hardcoded for B=2, S=2048, HID=4096, H=32, 8 cores.
"""

import math
from contextlib import ExitStack
from dataclasses import dataclass

import ml_dtypes
import numpy as np

import concourse.bacc as bacc
import concourse.mybir as mybir
import concourse.tile as tile
from concourse.bass import ts
from concourse.bass_utils import run_bass_kernel_spmd
from concourse.masks import make_identity

F32 = mybir.dt.float32
BF16 = mybir.dt.bfloat16
AF = mybir.ActivationFunctionType
ALU = mybir.AluOpType
AX = mybir.AxisListType
BF = ml_dtypes.bfloat16

N_CORES = 8


@dataclass(frozen=True)
class Cfg:
    B: int = 2
    S: int = 2048
    HID: int = 4096
    H_CORE: int = 4          # heads handled by this core
    HD: int = 128
    TOK_GROUP: int = 256     # phase-1 token group (SBUF-budget bound)

    @property
    def TOKS(self):
        return self.B * self.S

    @property
    def KT(self):
        return self.HID // 128          # hid tiles (contraction)

    @property
    def QK_CT(self):
        return 2 * self.H_CORE          # q+k coltiles

    @property
    def VC(self):
        return self.H_CORE * self.HD    # v columns (<= 512)

    @property
    def NQT(self):
        return self.S // 128            # q tiles per sequence

    @property
    def NPAIR(self):
        return self.B * self.H_CORE

    @property
    def MC(self):
        return self.VC // 128           # dense contraction chunks

    @property
    def N_DENSE(self):
        return min(2048, self.HID)      # dense output chunk (<=4 psum banks)


FULL = Cfg()


def input_specs(cfg: Cfg):
    c = cfg
    return {
        "hsT": ([c.HID, c.TOKS], BF16),
        "wqkv_qk": ([c.QK_CT, 128, c.HID], BF16),
        "wqkv_v": ([128, c.KT * c.VC], BF16),
        "bias_qk": ([128, c.QK_CT], F32),
        "bqkv_v_rep": ([128, c.VC], BF16),
        "alibi": ([c.NPAIR, c.S], F32),
        "neg_alibi": ([128, c.NPAIR * c.NQT], F32),
        "wd": ([c.MC * 128, c.HID], BF16),
    }


def output_specs(cfg: Cfg):
    return {"out_part": ([cfg.TOKS, cfg.HID], F32)}


def build(ctx: ExitStack, tc, outs, ins, cfg: Cfg):
    c = cfg
    nc = tc.nc
    hsT, wqkv_qk, wqkv_v = ins["hsT"], ins["wqkv_qk"], ins["wqkv_v"]
    bias_qk, bqkv_v_rep = ins["bias_qk"], ins["bqkv_v_rep"]
    alibi, neg_alibi, wd = ins["alibi"], ins["neg_alibi"], ins["wd"]
    out_part = outs["out_part"]

    TG = c.TOK_GROUP
    NG = c.TOKS // TG
    NB = TG // 512 if TG >= 512 else 1
    WB = min(512, TG)

    # ---- persistent SBUF ----
    persist = ctx.enter_context(tc.tile_pool(name="persist", bufs=1))
    qkt_sb = persist.tile([128, c.QK_CT, c.TOKS], BF16, tag="qkt")
    v_sb = persist.tile([128, c.TOKS // 128, c.VC], BF16, tag="v")
    bias_qk_sb = persist.tile([128, c.QK_CT], F32, tag="bias_qk")
    bvrep_sb = persist.tile([128, c.VC], BF16, tag="bvrep")
    negal_sb = persist.tile([128, c.NPAIR * c.NQT], F32, tag="negal")
    ident_sb = persist.tile([128, 128], BF16, tag="ident")
    mask_sb = persist.tile([128, 128], F32, tag="mask")

    nc.sync.dma_start(out=bias_qk_sb[:], in_=bias_qk[:])
    nc.sync.dma_start(out=bvrep_sb[:], in_=bqkv_v_rep[:])
    nc.sync.dma_start(out=negal_sb[:], in_=neg_alibi[:])
    make_identity(nc, ident_sb[:])
    # causal mask tile: row q, col k (within diagonal tile): q >= k keep 0
    nc.gpsimd.memset(mask_sb[:], 0.0)
    nc.gpsimd.affine_select(
        out=mask_sb[:], in_=mask_sb[:], compare_op=ALU.is_ge,
        fill=-1.0e30, base=0, pattern=[[-1, 128]], channel_multiplier=1,
    )

    # ================= Phase 1: QKV projection =================
    _sid1, _ = nc.enter_named_scope("p1_qkv", False)
    with (
        tc.tile_pool(name="p1_hs", bufs=2) as hs_pool,
        tc.tile_pool(name="p1_w", bufs=2) as w_pool,
        tc.tile_pool(name="p1_wv", bufs=1) as wv_pool,
        tc.tile_pool(name="p1_ps", bufs=2, space="PSUM") as ps_pool,
        tc.tile_pool(name="p1_psv", bufs=2, space="PSUM") as psv_pool,
    ):
        wqkv_v_sb = wv_pool.tile([128, c.KT * c.VC], BF16, tag="wqkv_v")
        nc.sync.dma_start(out=wqkv_v_sb[:], in_=wqkv_v[:])
        for g in range(NG):
            g0 = g * TG
            hs_sb = hs_pool.tile([128, c.KT, TG], BF16, tag="hs")
            for kt in range(c.KT):
                nc.sync.dma_start(
                    out=hs_sb[:, kt, :], in_=hsT[ts(kt, 128), g0:g0 + TG]
                )
            # Q^T / K^T: out [col, tok]
            for ct in range(c.QK_CT):
                wst = w_pool.tile([128, c.HID], BF16, tag="wstripe")
                nc.sync.dma_start(out=wst[:], in_=wqkv_qk[ct])
                qk_ps = ps_pool.tile([128, TG], F32, tag="qk_ps")
                for kt in range(c.KT):
                    for nb in range(NB):
                        nc.tensor.matmul(
                            qk_ps[:, ts(nb, WB)],
                            wst[:, ts(kt, 128)],
                            hs_sb[:, kt, ts(nb, WB)],
                            start=(kt == 0), stop=(kt == c.KT - 1),
                        )
                nc.vector.tensor_scalar(
                    qkt_sb[:, ct, g0:g0 + TG], qk_ps[:],
                    bias_qk_sb[:, ct:ct + 1], None, ALU.add,
                )
            # V: out [tok, vcol]
            for tt in range(TG // 128):
                v_ps = psv_pool.tile([128, c.VC], F32, tag="v_ps")
                for kt in range(c.KT):
                    nc.tensor.matmul(
                        v_ps[:],
                        hs_sb[:, kt, ts(tt, 128)],
                        wqkv_v_sb[:, ts(kt, c.VC)],
                        start=(kt == 0), stop=(kt == c.KT - 1),
                    )
                nc.vector.tensor_tensor(
                    v_sb[:, g0 // 128 + tt, :], v_ps[:], bvrep_sb[:], ALU.add
                )

    nc.leave_named_scope("p1_qkv", _sid1, False)

    # ================= Phase 2: attention =================
    _sid2, _ = nc.enter_named_scope("p2_attn", False)
    # ctxT allocated only now so phase 1 can use the SBUF it will occupy
    ctx_persist = ctx.enter_context(tc.tile_pool(name="ctx_persist", bufs=1))
    ctxT_sb = ctx_persist.tile([128, c.NPAIR, c.S], BF16, tag="ctxT")
    with (
        tc.tile_pool(name="a_rep", bufs=2) as rep_pool,
        tc.tile_pool(name="a_row", bufs=3) as row_pool,
        tc.tile_pool(name="a_sm", bufs=6) as sm_pool,
        tc.tile_pool(name="a_sps", bufs=3, space="PSUM") as sps_pool,
        tc.tile_pool(name="a_pt", bufs=2, space="PSUM") as pt_pool,
        tc.tile_pool(name="a_ctx", bufs=2, space="PSUM") as ctx_pool,
    ):
        for p in range(c.NPAIR):
            b, hl = divmod(p, c.H_CORE)
            qT = qkt_sb[:, hl, b * c.S:(b + 1) * c.S]
            kT = qkt_sb[:, c.H_CORE + hl, b * c.S:(b + 1) * c.S]
            arow = rep_pool.tile([1, c.S], F32, tag="arow")
            arep = rep_pool.tile([128, c.S], F32, tag="arep")
            nc.sync.dma_start(out=arow[:], in_=alibi[p:p + 1, :])
            nc.gpsimd.partition_broadcast(arep[:], arow[:])

            # software pipeline: stage A (scores+softmax) runs PIPE ahead of
            # stage B (transpose+PV) so PE never waits on DVE/ACT
            PIPE = 2
            pending = []

            def stage_a(qt, p=p, qT=qT, kT=kT, arep=arep):
                klen = (qt + 1) * 128
                nch = (klen + 511) // 512
                prow = row_pool.tile([128, c.S], BF16, tag="prow")
                sums = sm_pool.tile([128, 8], F32, tag="sums")
                for ch in range(nch):
                    w = min(512, klen - ch * 512)
                    s_ps = sps_pool.tile([128, 512], F32, tag="s_ps")
                    nc.tensor.matmul(
                        s_ps[:, :w], qT[:, ts(qt, 128)],
                        kT[:, ch * 512:ch * 512 + w],
                        start=True, stop=True,
                    )
                    s_sb = sm_pool.tile([128, 512], F32, tag="s_sb")
                    nc.vector.tensor_tensor(
                        s_sb[:, :w], s_ps[:, :w],
                        arep[:, ch * 512:ch * 512 + w], ALU.add
                    )
                    if ch == nch - 1:
                        nc.vector.tensor_tensor(
                            s_sb[:, w - 128:w], s_sb[:, w - 128:w],
                            mask_sb[:], ALU.add
                        )
                    nc.scalar.activation(
                        prow[:, ch * 512:ch * 512 + w], s_sb[:, :w], AF.Exp,
                        bias=negal_sb[:, p * c.NQT + qt:p * c.NQT + qt + 1],
                        scale=1.0, accum_out=sums[:, ch:ch + 1],
                    )
                stot = sm_pool.tile([128, 1], F32, tag="stot")
                recip = sm_pool.tile([128, 1], F32, tag="recip")
                nc.vector.reduce_sum(stot[:], sums[:, :nch], axis=AX.X)
                nc.vector.reciprocal(recip[:], stot[:])
                nc.vector.tensor_scalar(
                    prow[:, :klen], prow[:, :klen], recip[:, 0:1], None, ALU.mult
                )
                return prow

            def stage_b(qt, prow, p=p, b=b, hl=hl):
                ptrow = row_pool.tile([128, c.S], BF16, tag="ptrow")
                for kt in range(qt + 1):
                    pt_ps = pt_pool.tile([128, 128], BF16, tag="pt_ps")
                    nc.tensor.transpose(pt_ps[:], prow[:, ts(kt, 128)], ident_sb[:])
                    nc.vector.tensor_copy(ptrow[:, ts(kt, 128)], pt_ps[:])
                ctx_ps = ctx_pool.tile([128, 128], F32, tag="ctx_ps")
                for kt in range(qt + 1):
                    nc.tensor.matmul(
                        ctx_ps[:],
                        v_sb[:, b * c.NQT + kt, ts(hl, 128)],
                        ptrow[:, ts(kt, 128)],
                        start=(kt == 0), stop=(kt == qt),
                    )
                nc.scalar.copy(ctxT_sb[:, p, ts(qt, 128)], ctx_ps[:])

            for qt in range(c.NQT):
                pending.append((qt, stage_a(qt)))
                if len(pending) > PIPE:
                    q0, pr0 = pending.pop(0)
                    stage_b(q0, pr0)
            for q0, pr0 in pending:
                stage_b(q0, pr0)

    nc.leave_named_scope("p2_attn", _sid2, False)

    # ================= Phase 3: dense =================
    _sid3, _ = nc.enter_named_scope("p3_dense", False)
    with (
        tc.tile_pool(name="d_w", bufs=1) as dw_pool,
        tc.tile_pool(name="d_ps", bufs=2, space="PSUM") as dps_pool,
        tc.tile_pool(name="d_out", bufs=3) as dout_pool,
    ):
        wd_sb = dw_pool.tile([128, c.MC, c.HID], BF16, tag="wd")
        for mc in range(c.MC):
            nc.sync.dma_start(out=wd_sb[:, mc, :], in_=wd[ts(mc, 128), :])
        ND = c.N_DENSE
        for tt in range(c.TOKS // 128):
            b, st = divmod(tt, c.NQT)
            for nh in range(c.HID // ND):
                d_ps = dps_pool.tile([128, ND], F32, tag="d_ps")
                for mc in range(c.MC):
                    for nb in range(ND // 512):
                        nc.tensor.matmul(
                            d_ps[:, ts(nb, 512)],
                            ctxT_sb[:, b * c.H_CORE + mc, ts(st, 128)],
                            wd_sb[:, mc, nh * ND + nb * 512:nh * ND + (nb + 1) * 512],
                            start=(mc == 0), stop=(mc == c.MC - 1),
                        )
                o_sb = dout_pool.tile([128, ND], F32, tag="o_sb")
                nc.scalar.copy(o_sb[:], d_ps[:])
                nc.sync.dma_start(
                    out=out_part[ts(tt, 128), nh * ND:(nh + 1) * ND], in_=o_sb[:]
                )
    nc.leave_named_scope("p3_dense", _sid3, False)


# ================= host side =================

def prep_shared(hidden_states, cfg):
    """hsT [HID, TOKS] bf16 — shared across cores."""
    c = cfg
    hs = np.asarray(hidden_states, np.float32).reshape(c.TOKS, c.HID)
    return np.ascontiguousarray(hs.T).astype(BF)


def prep_core(alibi, Wqkv, bqkv, Wd, heads, cfg):
    """Per-core inputs for `heads` (list of H_CORE global head indices)."""
    c = cfg
    inv = 1.0 / math.sqrt(c.HD)
    Wq = np.asarray(Wqkv, np.float32).reshape(c.HID, -1, 3, c.HD)
    bq = np.asarray(bqkv, np.float32).reshape(-1, 3, c.HD)
    H = Wq.shape[1]

    # q cols pre-scaled by inv_norm; ct order: q heads then k heads
    w_q = Wq[:, heads, 0, :] * inv                      # [HID, H_CORE, HD]
    w_k = Wq[:, heads, 1, :]
    w_qk = np.concatenate([w_q, w_k], axis=1)           # [HID, QK_CT, 128]
    # -> [ct][hid_p][kt*128+col]: SBUF stripe rows are hid-within-chunk
    w_qk = w_qk.reshape(c.KT, 128, c.QK_CT, 128).transpose(2, 1, 0, 3)
    wqkv_qk = np.ascontiguousarray(w_qk.reshape(c.QK_CT, 128, c.HID)).astype(BF)

    w_v = Wq[:, heads, 2, :].reshape(c.HID, c.VC)       # [HID, VC]
    w_v = w_v.reshape(c.KT, 128, c.VC).transpose(1, 0, 2)  # [p, kt, vc]
    wqkv_v = np.ascontiguousarray(w_v.reshape(128, c.KT * c.VC)).astype(BF)

    b_q = bq[heads, 0, :] * inv                         # [H_CORE, 128]
    b_k = bq[heads, 1, :]
    b_qk = np.concatenate([b_q, b_k], axis=0)           # [QK_CT, 128]
    bias_qk = np.ascontiguousarray(b_qk.T).astype(np.float32)  # [128, QK_CT]

    b_v = bq[heads, 2, :].reshape(c.VC)
    bqkv_v_rep = np.ascontiguousarray(
        np.broadcast_to(b_v[None, :], (128, c.VC))
    ).astype(BF)

    al = np.asarray(alibi, np.float32).reshape(c.B, H, c.S)[:, heads]  # [B,HC,S]
    alibi_c = np.ascontiguousarray(al.reshape(c.NPAIR, c.S)).astype(np.float32)
    negal = (-alibi_c).reshape(c.NPAIR, c.NQT, 128).transpose(2, 0, 1)
    neg_alibi = np.ascontiguousarray(
        negal.reshape(128, c.NPAIR * c.NQT)
    ).astype(np.float32)

    wd_c = np.asarray(Wd, np.float32).reshape(H, c.HD, c.HID)[heads]
    wd = np.ascontiguousarray(wd_c.reshape(c.MC * 128, c.HID)).astype(BF)

    return {
        "wqkv_qk": wqkv_qk,
        "wqkv_v": wqkv_v,
        "bias_qk": bias_qk,
        "bqkv_v_rep": bqkv_v_rep,
        "alibi": alibi_c,
        "neg_alibi": neg_alibi,
        "wd": wd,
    }


def build_nc(cfg, debug=False):
    nc = bacc.Bacc("TRN2", target_bir_lowering=False, debug=debug)
    ins = {
        n: nc.dram_tensor(n, sh, dt, kind="ExternalInput").ap()
        for n, (sh, dt) in input_specs(cfg).items()
    }
    outs = {
        n: nc.dram_tensor(n, sh, dt, kind="ExternalOutput").ap()
        for n, (sh, dt) in output_specs(cfg).items()
    }
    with tile.TileContext(nc) as tc:
        with ExitStack() as es:
            build(es, tc, outs, ins, cfg)
    nc.compile()
    return nc


_NC_CACHE = {}


def _get_nc(cfg):
    if cfg not in _NC_CACHE:
        _NC_CACHE[cfg] = build_nc(cfg)
    return _NC_CACHE[cfg]


def _run(inputs, trace=False, **kwargs):
    cfg = FULL
    c = cfg
    hidden_states = np.asarray(inputs["hidden_states"], np.float32)
    residual = np.asarray(inputs["residual"], np.float32)
    alibi = np.asarray(inputs["alibi"], np.float32)
    Wqkv = np.asarray(inputs["Wqkv"], np.float32)
    bqkv = np.asarray(inputs["bqkv"], np.float32)
    Wd = np.asarray(inputs["Wd"], np.float32)
    bd = np.asarray(inputs["bd"], np.float32)
    H = Wqkv.shape[1] // (3 * c.HD)

    nc = _get_nc(cfg)
    hsT = prep_shared(hidden_states, cfg)
    in_maps = []
    for core in range(N_CORES):
        heads = list(range(core * c.H_CORE, (core + 1) * c.H_CORE))
        m = {"hsT": hsT}
        m.update(prep_core(alibi, Wqkv, bqkv, Wd, heads, cfg))
        in_maps.append(m)

    res = run_bass_kernel_spmd(
        nc, in_maps, core_ids=list(range(N_CORES)), trace=trace, **kwargs
    )
    acc = np.zeros((c.TOKS, c.HID), np.float64)
    for r in res.results:
        acc += r["out_part"].astype(np.float64)
    out = acc.reshape(c.B, c.S, c.HID) + residual.astype(np.float64) + bd
    return out.astype(np.float32), res


def kernel(**inputs):
    out, _ = _run(inputs, trace=False)
    return out


# revision 13
# speedup vs baseline: 1.1117x; 1.1117x over previous
"""BLOOM attention block on 8 TRN2 NeuronCores.

Tensor-parallel over heads: core c computes heads 4c..4c+3 for both batches
(8 (b,h) pairs/core). Device math in bf16 with fp32 accumulation:

  phase 1: Q^T/K^T = (Wqkv_qk^T h)  [head-dim on partitions], V = h Wqkv_v
  phase 2: per pair, causal-tiled scores + alibi softmax (stabilized with the
           analytic bias -alibi[q] instead of a max pass), P^T via PE
           transpose, ctx^T = V^T P accumulated in dense-ready layout
  phase 3: out_part = ctx Wd_c (per-core partial, fp32)

Host: shards/casts inputs, then out = residual + bd + sum_c out_part_c.
Self-contained: shapes hardcoded for B=2, S=2048, HID=4096, H=32, 8 cores.
"""

import math
from contextlib import ExitStack
from dataclasses import dataclass

import ml_dtypes
import numpy as np

import concourse.bacc as bacc
import concourse.mybir as mybir
import concourse.tile as tile
from concourse.bass import ts
from concourse.bass_utils import run_bass_kernel_spmd
from concourse.masks import make_identity

F32 = mybir.dt.float32
BF16 = mybir.dt.bfloat16
AF = mybir.ActivationFunctionType
ALU = mybir.AluOpType
AX = mybir.AxisListType
BF = ml_dtypes.bfloat16

N_CORES = 8


@dataclass(frozen=True)
class Cfg:
    B: int = 2
    S: int = 2048
    HID: int = 4096
    H_CORE: int = 4          # heads handled by this core
    HD: int = 128
    TOK_GROUP: int = 256     # phase-1 token group (SBUF-budget bound)

    @property
    def TOKS(self):
        return self.B * self.S

    @property
    def KT(self):
        return self.HID // 128          # hid tiles (contraction)

    @property
    def QK_CT(self):
        return 2 * self.H_CORE          # q+k coltiles

    @property
    def VC(self):
        return self.H_CORE * self.HD    # v columns (<= 512)

    @property
    def NQT(self):
        return self.S // 128            # q tiles per sequence

    @property
    def NPAIR(self):
        return self.B * self.H_CORE

    @property
    def MC(self):
        return self.VC // 128           # dense contraction chunks

    @property
    def NCH(self):
        return (self.S + 511) // 512    # max score chunks per q row


FULL = Cfg()


def input_specs(cfg: Cfg):
    c = cfg
    return {
        "hsT": ([c.HID, c.TOKS], BF16),
        "wqkv_qk": ([c.QK_CT, 128, c.HID], BF16),
        "wqkv_v": ([128, c.KT * c.VC], BF16),
        "bias_qk": ([128, c.QK_CT], F32),
        "bqkv_v_rep": ([128, c.VC], BF16),
        "ramp": ([128, 512], F32),
        "slopes": ([128, c.NPAIR], F32),
        "bias_qc": ([128, c.NPAIR * c.NQT * c.NCH], F32),
        "wd": ([c.MC * 128, c.HID], BF16),
    }


def output_specs(cfg: Cfg):
    return {"out_part": ([cfg.TOKS, cfg.HID], F32)}


def build(ctx: ExitStack, tc, outs, ins, cfg: Cfg):
    c = cfg
    nc = tc.nc
    hsT, wqkv_qk, wqkv_v = ins["hsT"], ins["wqkv_qk"], ins["wqkv_v"]
    bias_qk, bqkv_v_rep = ins["bias_qk"], ins["bqkv_v_rep"]
    ramp, slopes, bias_qc, wd = ins["ramp"], ins["slopes"], ins["bias_qc"], ins["wd"]
    out_part = outs["out_part"]

    TG = c.TOK_GROUP
    NG = c.TOKS // TG
    NB = TG // 512 if TG >= 512 else 1
    WB = min(512, TG)

    # ---- persistent SBUF ----
    persist = ctx.enter_context(tc.tile_pool(name="persist", bufs=1))
    qkt_sb = persist.tile([128, c.QK_CT, c.TOKS], BF16, tag="qkt")
    v_sb = persist.tile([128, c.TOKS // 128, c.VC], BF16, tag="v")
    bias_qk_sb = persist.tile([128, c.QK_CT], F32, tag="bias_qk")
    bvrep_sb = persist.tile([128, c.VC], BF16, tag="bvrep")
    ramp_sb = persist.tile([128, 512], F32, tag="ramp")
    slopes_sb = persist.tile([128, c.NPAIR], F32, tag="slopes")
    bias_qc_sb = persist.tile([128, c.NPAIR * c.NQT * c.NCH], F32, tag="bias_qc")
    ident_sb = persist.tile([128, 128], BF16, tag="ident")
    mask_sb = persist.tile([128, 128], F32, tag="mask")

    nc.sync.dma_start(out=bias_qk_sb[:], in_=bias_qk[:])
    nc.sync.dma_start(out=bvrep_sb[:], in_=bqkv_v_rep[:])
    nc.sync.dma_start(out=ramp_sb[:], in_=ramp[:])
    nc.sync.dma_start(out=slopes_sb[:], in_=slopes[:])
    nc.sync.dma_start(out=bias_qc_sb[:], in_=bias_qc[:])
    make_identity(nc, ident_sb[:])
    # causal mask tile: row q, col k (within diagonal tile): q >= k keep 0
    nc.gpsimd.memset(mask_sb[:], 0.0)
    nc.gpsimd.affine_select(
        out=mask_sb[:], in_=mask_sb[:], compare_op=ALU.is_ge,
        fill=-1.0e30, base=0, pattern=[[-1, 128]], channel_multiplier=1,
    )

    # ================= Phase 1: QKV projection =================
    _sid1, _ = nc.enter_named_scope("p1_qkv", False)
    with (
        tc.tile_pool(name="p1_hs", bufs=2) as hs_pool,
        tc.tile_pool(name="p1_w", bufs=2) as w_pool,
        tc.tile_pool(name="p1_wv", bufs=1) as wv_pool,
        tc.tile_pool(name="p1_ps", bufs=2, space="PSUM") as ps_pool,
        tc.tile_pool(name="p1_psv", bufs=2, space="PSUM") as psv_pool,
    ):
        wqkv_v_sb = wv_pool.tile([128, c.KT * c.VC], BF16, tag="wqkv_v")
        nc.sync.dma_start(out=wqkv_v_sb[:], in_=wqkv_v[:])
        for g in range(NG):
            g0 = g * TG
            hs_sb = hs_pool.tile([128, c.KT, TG], BF16, tag="hs")
            for kt in range(c.KT):
                nc.sync.dma_start(
                    out=hs_sb[:, kt, :], in_=hsT[ts(kt, 128), g0:g0 + TG]
                )
            # Q^T / K^T: out [col, tok]
            for ct in range(c.QK_CT):
                wst = w_pool.tile([128, c.HID], BF16, tag="wstripe")
                nc.sync.dma_start(out=wst[:], in_=wqkv_qk[ct])
                qk_ps = ps_pool.tile([128, TG], F32, tag="qk_ps")
                for kt in range(c.KT):
                    for nb in range(NB):
                        nc.tensor.matmul(
                            qk_ps[:, ts(nb, WB)],
                            wst[:, ts(kt, 128)],
                            hs_sb[:, kt, ts(nb, WB)],
                            start=(kt == 0), stop=(kt == c.KT - 1),
                        )
                nc.vector.tensor_scalar(
                    qkt_sb[:, ct, g0:g0 + TG], qk_ps[:],
                    bias_qk_sb[:, ct:ct + 1], None, ALU.add,
                )
            # V: out [tok, vcol]
            for tt in range(TG // 128):
                v_ps = psv_pool.tile([128, c.VC], F32, tag="v_ps")
                for kt in range(c.KT):
                    nc.tensor.matmul(
                        v_ps[:],
                        hs_sb[:, kt, ts(tt, 128)],
                        wqkv_v_sb[:, ts(kt, c.VC)],
                        start=(kt == 0), stop=(kt == c.KT - 1),
                    )
                nc.vector.tensor_tensor(
                    v_sb[:, g0 // 128 + tt, :], v_ps[:], bvrep_sb[:], ALU.add
                )
    nc.leave_named_scope("p1_qkv", _sid1, False)

    # ============ Phase 2+3: attention fused with dense ============
    # dense matmuls interleave with attention so the PE stays HAM-warm
    # (transpose-mode matmuls do not count as PE activity for HAM)
    _sid2, _ = nc.enter_named_scope("p23_attn_dense", False)
    wd_persist = ctx.enter_context(tc.tile_pool(name="wd_persist", bufs=1))
    wd_sb = wd_persist.tile([128, c.MC, c.HID], BF16, tag="wd")
    for mc in range(c.MC):
        nc.sync.dma_start(out=wd_sb[:, mc, :], in_=wd[ts(mc, 128), :])
    with (
        tc.tile_pool(name="a_row", bufs=1) as row_pool,
        tc.tile_pool(name="a_sm", bufs=1) as sm_pool,
        tc.tile_pool(name="d_out", bufs=1) as dout_pool,
        tc.tile_pool(name="a_sps", bufs=2, space="PSUM") as sps_pool,
        tc.tile_pool(name="a_pt", bufs=2, space="PSUM") as pt_pool,
        tc.tile_pool(name="a_ctx", bufs=2, space="PSUM") as ctx_pool,
        tc.tile_pool(name="d_ps", bufs=2, space="PSUM") as dps_pool,
    ):
        def stage_a(b, hl, qt):
            p = b * c.H_CORE + hl
            klen = (qt + 1) * 128
            nch = (klen + 511) // 512
            qT = qkt_sb[:, hl, b * c.S:(b + 1) * c.S]
            kT = qkt_sb[:, c.H_CORE + hl, b * c.S:(b + 1) * c.S]
            prow = row_pool.tile([128, c.S], BF16, tag="prow", bufs=2 * c.H_CORE)
            sums = sm_pool.tile([128, c.NCH], F32, tag="sums", bufs=2 * c.H_CORE)
            for ch in range(nch):
                w = min(512, klen - ch * 512)
                s_ps = sps_pool.tile([128, 512], F32, tag="s_ps")
                nc.tensor.matmul(
                    s_ps[:, :w], qT[:, ts(qt, 128)],
                    kT[:, ch * 512:ch * 512 + w],
                    start=True, stop=True,
                )
                s_sb = sm_pool.tile([128, 512], F32, tag="s_sb", bufs=4)
                # s = scores + slope*ramp  (local alibi; offset goes in exp bias)
                nc.vector.scalar_tensor_tensor(
                    s_sb[:, :w], ramp_sb[:, :w], slopes_sb[:, p:p + 1],
                    s_ps[:, :w], ALU.mult, ALU.add,
                )
                if ch == nch - 1:
                    nc.vector.tensor_tensor(
                        s_sb[:, w - 128:w], s_sb[:, w - 128:w],
                        mask_sb[:], ALU.add,
                    )
                bidx = (p * c.NQT + qt) * c.NCH + ch
                nc.scalar.activation(
                    prow[:, ch * 512:ch * 512 + w], s_sb[:, :w], AF.Exp,
                    bias=bias_qc_sb[:, bidx:bidx + 1],
                    scale=1.0, accum_out=sums[:, ch:ch + 1],
                )
            stot = sm_pool.tile([128, 1], F32, tag="stot", bufs=4)
            recip = sm_pool.tile([128, 1], F32, tag="recip", bufs=4)
            nc.vector.reduce_sum(stot[:], sums[:, :nch], axis=AX.X)
            nc.vector.reciprocal(recip[:], stot[:])
            nc.vector.tensor_scalar(
                prow[:, :klen], prow[:, :klen], recip[:, 0:1], None, ALU.mult
            )
            return prow

        def stage_b(b, hl, qt, prow, ctx_row):
            ptrow = row_pool.tile([128, c.S], BF16, tag="ptrow", bufs=3)
            for kt in range(qt + 1):
                pt_ps = pt_pool.tile([128, 128], BF16, tag="pt_ps")
                nc.tensor.transpose(pt_ps[:], prow[:, ts(kt, 128)], ident_sb[:])
                nc.vector.tensor_copy(ptrow[:, ts(kt, 128)], pt_ps[:])
            ctx_ps = ctx_pool.tile([128, 128], F32, tag="ctx_ps")
            for kt in range(qt + 1):
                nc.tensor.matmul(
                    ctx_ps[:],
                    v_sb[:, b * c.NQT + kt, ts(hl, 128)],
                    ptrow[:, ts(kt, 128)],
                    start=(kt == 0), stop=(kt == qt),
                )
            nc.scalar.copy(ctx_row[:, hl, :], ctx_ps[:])

        def flush(b, qt, prows):
            ctx_row = row_pool.tile([128, c.MC, 128], BF16, tag="ctx_row", bufs=2)
            for hl in range(c.H_CORE):
                stage_b(b, hl, qt, prows[hl], ctx_row)
            tt = b * c.NQT + qt
            for nb in range(c.HID // 512):
                d_ps = dps_pool.tile([128, 512], F32, tag="d_ps")
                for mc in range(c.MC):
                    nc.tensor.matmul(
                        d_ps[:],
                        ctx_row[:, mc, :],
                        wd_sb[:, mc, ts(nb, 512)],
                        start=(mc == 0), stop=(mc == c.MC - 1),
                    )
                o_sb = dout_pool.tile([128, 512], F32, tag="o_sb", bufs=3)
                nc.scalar.copy(o_sb[:], d_ps[:])
                nc.sync.dma_start(
                    out=out_part[ts(tt, 128), ts(nb, 512)], in_=o_sb[:]
                )

        for b in range(c.B):
            prev = None
            for qt in range(c.NQT):
                cur = [stage_a(b, hl, qt) for hl in range(c.H_CORE)]
                if prev is not None:
                    flush(b, prev[0], prev[1])
                prev = (qt, cur)
            flush(b, prev[0], prev[1])
    nc.leave_named_scope("p23_attn_dense", _sid2, False)


# ================= host side =================

def prep_shared(hidden_states, cfg):
    """hsT [HID, TOKS] bf16 — shared across cores."""
    c = cfg
    hs = np.asarray(hidden_states, np.float32).reshape(c.TOKS, c.HID)
    return np.ascontiguousarray(hs.T).astype(BF)


def prep_core(alibi, Wqkv, bqkv, Wd, heads, cfg):
    """Per-core inputs for `heads` (list of H_CORE global head indices)."""
    c = cfg
    inv = 1.0 / math.sqrt(c.HD)
    Wq = np.asarray(Wqkv, np.float32).reshape(c.HID, -1, 3, c.HD)
    bq = np.asarray(bqkv, np.float32).reshape(-1, 3, c.HD)
    H = Wq.shape[1]

    # q cols pre-scaled by inv_norm; ct order: q heads then k heads
    w_q = Wq[:, heads, 0, :] * inv                      # [HID, H_CORE, HD]
    w_k = Wq[:, heads, 1, :]
    w_qk = np.concatenate([w_q, w_k], axis=1)           # [HID, QK_CT, 128]
    # -> [ct][hid_p][kt*128+col]: SBUF stripe rows are hid-within-chunk
    w_qk = w_qk.reshape(c.KT, 128, c.QK_CT, 128).transpose(2, 1, 0, 3)
    wqkv_qk = np.ascontiguousarray(w_qk.reshape(c.QK_CT, 128, c.HID)).astype(BF)

    w_v = Wq[:, heads, 2, :].reshape(c.HID, c.VC)       # [HID, VC]
    w_v = w_v.reshape(c.KT, 128, c.VC).transpose(1, 0, 2)  # [p, kt, vc]
    wqkv_v = np.ascontiguousarray(w_v.reshape(128, c.KT * c.VC)).astype(BF)

    b_q = bq[heads, 0, :] * inv                         # [H_CORE, 128]
    b_k = bq[heads, 1, :]
    b_qk = np.concatenate([b_q, b_k], axis=0)           # [QK_CT, 128]
    bias_qk = np.ascontiguousarray(b_qk.T).astype(np.float32)  # [128, QK_CT]

    b_v = bq[heads, 2, :].reshape(c.VC)
    bqkv_v_rep = np.ascontiguousarray(
        np.broadcast_to(b_v[None, :], (128, c.VC))
    ).astype(BF)

    al = np.asarray(alibi, np.float32).reshape(c.B, H, c.S)[:, heads]  # [B,HC,S]
    alibi_c = al.reshape(c.NPAIR, c.S).astype(np.float32)
    ramp = np.ascontiguousarray(
        np.broadcast_to(np.arange(512, dtype=np.float32)[None, :], (128, 512))
    )
    slope = alibi_c[:, 1] - alibi_c[:, 0]                  # [NPAIR]
    slopes = np.ascontiguousarray(
        np.broadcast_to(slope[None, :], (128, c.NPAIR))
    ).astype(np.float32)
    # exp bias per (pair, qt, chunk): alibi[c*512] - alibi[q]
    bias_qc = np.zeros((128, c.NPAIR, c.NQT, c.NCH), np.float32)
    for p in range(c.NPAIR):
        for qt in range(c.NQT):
            nch = ((qt + 1) * 128 + 511) // 512
            qpos = qt * 128 + np.arange(128)
            for ch in range(nch):
                bias_qc[:, p, qt, ch] = alibi_c[p, ch * 512] - alibi_c[p, qpos]
    bias_qc = np.ascontiguousarray(
        bias_qc.reshape(128, c.NPAIR * c.NQT * c.NCH)
    )

    wd_c = np.asarray(Wd, np.float32).reshape(H, c.HD, c.HID)[heads]
    wd = np.ascontiguousarray(wd_c.reshape(c.MC * 128, c.HID)).astype(BF)

    return {
        "wqkv_qk": wqkv_qk,
        "wqkv_v": wqkv_v,
        "bias_qk": bias_qk,
        "bqkv_v_rep": bqkv_v_rep,
        "ramp": ramp,
        "slopes": slopes,
        "bias_qc": bias_qc,
        "wd": wd,
    }


def build_nc(cfg, debug=False):
    nc = bacc.Bacc("TRN2", target_bir_lowering=False, debug=debug)
    ins = {
        n: nc.dram_tensor(n, sh, dt, kind="ExternalInput").ap()
        for n, (sh, dt) in input_specs(cfg).items()
    }
    outs = {
        n: nc.dram_tensor(n, sh, dt, kind="ExternalOutput").ap()
        for n, (sh, dt) in output_specs(cfg).items()
    }
    with tile.TileContext(nc) as tc:
        with ExitStack() as es:
            build(es, tc, outs, ins, cfg)
    nc.compile()
    return nc


_NC_CACHE = {}


def _get_nc(cfg):
    if cfg not in _NC_CACHE:
        _NC_CACHE[cfg] = build_nc(cfg)
    return _NC_CACHE[cfg]


def _run(inputs, trace=False, **kwargs):
    cfg = FULL
    c = cfg
    hidden_states = np.asarray(inputs["hidden_states"], np.float32)
    residual = np.asarray(inputs["residual"], np.float32)
    alibi = np.asarray(inputs["alibi"], np.float32)
    Wqkv = np.asarray(inputs["Wqkv"], np.float32)
    bqkv = np.asarray(inputs["bqkv"], np.float32)
    Wd = np.asarray(inputs["Wd"], np.float32)
    bd = np.asarray(inputs["bd"], np.float32)
    H = Wqkv.shape[1] // (3 * c.HD)

    nc = _get_nc(cfg)
    hsT = prep_shared(hidden_states, cfg)
    in_maps = []
    for core in range(N_CORES):
        heads = list(range(core * c.H_CORE, (core + 1) * c.H_CORE))
        m = {"hsT": hsT}
        m.update(prep_core(alibi, Wqkv, bqkv, Wd, heads, cfg))
        in_maps.append(m)

    res = run_bass_kernel_spmd(
        nc, in_maps, core_ids=list(range(N_CORES)), trace=trace, **kwargs
    )
    acc = np.zeros((c.TOKS, c.HID), np.float64)
    for r in res.results:
        acc += r["out_part"].astype(np.float64)
    out = acc.reshape(c.B, c.S, c.HID) + residual.astype(np.float64) + bd
    return out.astype(np.float32), res


def kernel(**inputs):
    out, _ = _run(inputs, trace=False)
    return out
